# revision 8
# baseline (speedup 1.0000x reference)
"""BertSelfAttention on 8 Trainium2 NeuronCores (Bass/Tile, SPMD, no collectives).

Problem: hidden_states [2, 2048, 1024], 16 heads x 64 dims, causal_bias added
along the key axis before softmax.

Sharding: core c handles batch b = c//4 and head-group g = c%4 (4 heads, i.e.
256 of the 1024 projection dims).  Pure SPMD - every core runs the same
program on its own slice; the host does the (free) slicing / transposes and
the final gather.

Per-core device algorithm (all matmuls in fp32r = full-rate fp32):
  QT[m, s] = Wq_g @ hsT + bq   (m = 256 local head dims, s = 2048 positions)
  KT[m, s] = Wk_g @ hsT + bk
  V [s, m] = (hs @ Wv_g.T) * expb[s]   (expb = exp(causal_bias), no bv)
  per head h (2 row-packed pairs):
    sT[k, sq]  = KT_h.T @ QT_h          (scores transposed, k = key pos)
    P [k, sq]  = exp(sT * 0.125)        (bias folded in via expb; no max
                                         subtraction needed: |s/8| < ~3)
    ctxu[65, sq] += [V'_h | expb].T @ P (rows 0..63 = unnormalized ctx^T,
                                         row 64 = softmax denominator)
  DMA ctxu to DRAM.
Host: ctx = (ctxu[:64] / ctxu[64]).T + bv  and scatter into [B, S, H].

The exp(bias) folding works because softmax(s + cb)_k = exp(s_k)*exp(cb_k) /
sum_k' exp(s_k')*exp(cb_k'), so scaling V rows and the denominator by
exp(cb_k) is exactly the bias add.
"""

import numpy as np

import concourse.bass as bass  # noqa: F401  (bass types via tile/bacc)
import concourse.tile as tile
from concourse import bacc, bass_utils, mybir

F32 = mybir.dt.float32
F32R = mybir.dt.float32r
AF = mybir.ActivationFunctionType

B, S, H = 2, 2048, 1024
NH, HD = 16, 64
M = 256          # per-core projection dims (4 heads)
KC = H // 128    # 8 contraction chunks for the projections
ST = S // 128    # 16 key-position chunks
N_CORES = 8

_NC_CACHE = {}


def _attention_kernel(tc):
    nc = tc.nc
    hsT = nc.dram_tensor("hsT", [H, S], F32R, kind="ExternalInput").ap()
    WqT = nc.dram_tensor("WqT", [H, M], F32R, kind="ExternalInput").ap()
    WkT = nc.dram_tensor("WkT", [H, M], F32R, kind="ExternalInput").ap()
    WvT = nc.dram_tensor("WvT", [H, M], F32R, kind="ExternalInput").ap()
    bq = nc.dram_tensor("bq", [128, 2], F32, kind="ExternalInput").ap()
    bk = nc.dram_tensor("bk", [128, 2], F32, kind="ExternalInput").ap()
    expb = nc.dram_tensor("expb", [128, ST], F32, kind="ExternalInput").ap()
    ctxu = nc.dram_tensor("ctxu", [4, HD + 1, S], F32, kind="ExternalOutput").ap()

    with (
        tc.tile_pool(name="const", bufs=1) as const,
        tc.tile_pool(name="big", bufs=1) as big,
    ):
        expb_sb = const.tile([128, ST], F32, tag="expb", name="expb")
        nc.sync.dma_start(out=expb_sb[:], in_=expb[:])
        bq_sb = const.tile([128, 2], F32, tag="bq", name="bq")
        nc.sync.dma_start(out=bq_sb[:], in_=bq[:])
        bk_sb = const.tile([128, 2], F32, tag="bk", name="bk")
        nc.sync.dma_start(out=bk_sb[:], in_=bk[:])
        ones_sb = const.tile([128, 4], F32, tag="ones", name="ones")
        nc.vector.memset(ones_sb[:], 1.0)

        hsT_t = []
        for k in range(KC):
            t = big.tile([128, S], F32R, tag=f"hsT{k}", name=f"hsT{k}")
            nc.sync.dma_start(out=t[:], in_=hsT[k * 128:(k + 1) * 128, :])
            hsT_t.append(t)
        wq_t, wk_t, wv_t = [], [], []
        for w_dram, lst, nm in ((WqT, wq_t, "wq"), (WkT, wk_t, "wk"), (WvT, wv_t, "wv")):
            for k in range(KC):
                t = big.tile([128, M], F32R, tag=f"{nm}{k}", name=f"{nm}{k}")
                nc.sync.dma_start(out=t[:], in_=w_dram[k * 128:(k + 1) * 128, :])
                lst.append(t)

        # Persistent projection outputs.
        QT = [big.tile([128, S], F32R, tag=f"QT{t}", name=f"QT{t}") for t in range(2)]
        KT = [big.tile([128, S], F32R, tag=f"KT{t}", name=f"KT{t}") for t in range(2)]
        # V' with exp(bias) column interleaved: per key chunk, 4 head blocks
        # of [64 scaled V dims | expb] = 260 columns.
        Vp = [big.tile([128, 4, HD + 1], F32R, tag=f"Vp{s}", name=f"Vp{s}") for s in range(ST)]

        with tc.tile_pool(name="pp", bufs=4, space="PSUM") as pp:
            # Q/K projections: out[m-tile, s-chunk] accumulated over H chunks.
            for w_t, out_t, bias_sb in ((wq_t, QT, bq_sb), (wk_t, KT, bk_sb)):
                for mt in range(2):
                    for sc in range(4):
                        ps = pp.tile([128, 512], F32, tag="qk", name="qk")
                        for k in range(KC):
                            nc.tensor.matmul(
                                ps[:],
                                w_t[k][:, mt * 128:(mt + 1) * 128],
                                hsT_t[k][:, sc * 512:(sc + 1) * 512],
                                start=(k == 0),
                                stop=(k == KC - 1),
                            )
                        nc.vector.tensor_scalar_add(
                            out_t[mt][:, sc * 512:(sc + 1) * 512],
                            ps[:],
                            bias_sb[:, mt:mt + 1],
                        )
            # V projection -> scale by expb -> interleaved V' layout.
            for st in range(ST):
                ps = pp.tile([128, M], F32, tag="v", name="v")
                for k in range(KC):
                    nc.tensor.matmul(
                        ps[:],
                        hsT_t[k][:, st * 128:(st + 1) * 128],
                        wv_t[k][:],
                        start=(k == 0),
                        stop=(k == KC - 1),
                    )
                nc.vector.tensor_scalar_mul(
                    Vp[st][:, :, 0:HD],
                    ps[:].rearrange("p (h d) -> p h d", h=4),
                    expb_sb[:, st:st + 1],
                )
                nc.vector.tensor_scalar_mul(
                    Vp[st][:, :, HD:HD + 1],
                    ones_sb[:].rearrange("p (h d) -> p h d", h=4),
                    expb_sb[:, st:st + 1],
                )

        # Attention, head-pair at a time (pair p = local heads 2p, 2p+1 living
        # on SBUF partitions 0-63 / 64-127 of QT[p]/KT[p] - row-packed on PE).
        with (
            tc.tile_pool(name="pt", bufs=2) as pt_pool,
            tc.tile_pool(name="cs", bufs=2) as cs_pool,
            tc.tile_pool(name="sc", bufs=1, space="PSUM") as sc_pool,
            tc.tile_pool(name="cx", bufs=2, space="PSUM") as cx_pool,
        ):
            for p in range(2):
                for sqc in range(4):
                    sq = slice(sqc * 512, (sqc + 1) * 512)
                    cA = cx_pool.tile([HD + 1, 512], F32, tag="cA", name="cA")
                    cB = cx_pool.tile([HD + 1, 512], F32, tag="cB", name="cB")
                    for kk in range(8):
                        sA = sc_pool.tile([128, 1024], F32, tag="sA", name="sA")
                        sB = sc_pool.tile([128, 1024], F32, tag="sB", name="sB")
                        for i in range(2):
                            kch = 2 * kk + i
                            ks = slice(kch * 128, (kch + 1) * 128)
                            nc.tensor.matmul(
                                sA[:, i * 512:(i + 1) * 512],
                                KT[p][0:64, ks],
                                QT[p][0:64, sq],
                            )
                            nc.tensor.matmul(
                                sB[:, i * 512:(i + 1) * 512],
                                KT[p][64:128, ks],
                                QT[p][64:128, sq],
                            )
                        pA = pt_pool.tile([128, 1024], F32R, tag="pA", name="pA")
                        pB = pt_pool.tile([128, 1024], F32R, tag="pB", name="pB")
                        nc.scalar.activation(pA[:], sA[:], AF.Exp, scale=0.125)
                        nc.scalar.activation(pB[:], sB[:], AF.Exp, scale=0.125)
                        for i in range(2):
                            kch = 2 * kk + i
                            flags = dict(
                                start=(kk == 0 and i == 0),
                                stop=(kk == 7 and i == 1),
                            )
                            nc.tensor.matmul(
                                cA[:],
                                Vp[kch][:, 2 * p, :],
                                pA[:, i * 512:(i + 1) * 512],
                                **flags,
                            )
                            nc.tensor.matmul(
                                cB[:],
                                Vp[kch][:, 2 * p + 1, :],
                                pB[:, i * 512:(i + 1) * 512],
                                **flags,
                            )
                    oA = cs_pool.tile([HD + 1, 512], F32, tag="oA", name="oA")
                    oB = cs_pool.tile([HD + 1, 512], F32, tag="oB", name="oB")
                    nc.vector.tensor_copy(oA[:], cA[:])
                    nc.vector.tensor_copy(oB[:], cB[:])
                    nc.sync.dma_start(out=ctxu[2 * p, :, sq], in_=oA[:])
                    nc.sync.dma_start(out=ctxu[2 * p + 1, :, sq], in_=oB[:])


def _head_slices(p):
    # Head h of pair p uses V' block index: pair 0 -> blocks 0,1; pair 1 -> 2,3.
    return 2 * p, 2 * p + 1


def build_nc():
    if "nc" in _NC_CACHE:
        return _NC_CACHE["nc"]
    nc = bacc.Bacc("TRN2", target_bir_lowering=False, debug=False)
    with tile.TileContext(nc) as tc:
        _attention_kernel(tc)
    nc.compile()
    _NC_CACHE["nc"] = nc
    return nc


def make_in_maps(hidden_states, causal_bias, Wq, bq, Wk, bk, Wv, bv):
    hs = np.ascontiguousarray(np.asarray(hidden_states, dtype=np.float32))
    cb = np.asarray(causal_bias, dtype=np.float32)
    expb = np.exp(cb).reshape(ST, 128).T.copy()  # [128, ST]
    hsT = [np.ascontiguousarray(hs[b].T) for b in range(B)]
    in_maps = []
    for c in range(N_CORES):
        b, g = divmod(c, 4)
        sl = slice(g * M, (g + 1) * M)
        in_maps.append({
            "hsT": hsT[b],
            "WqT": np.ascontiguousarray(np.asarray(Wq, np.float32)[sl].T),
            "WkT": np.ascontiguousarray(np.asarray(Wk, np.float32)[sl].T),
            "WvT": np.ascontiguousarray(np.asarray(Wv, np.float32)[sl].T),
            "bq": np.asarray(bq, np.float32)[sl].reshape(2, 128).T.copy(),
            "bk": np.asarray(bk, np.float32)[sl].reshape(2, 128).T.copy(),
            "expb": expb,
        })
    return in_maps


def gather_output(results, bv):
    bv = np.asarray(bv, np.float32)
    out = np.empty((B, S, H), np.float32)
    for c in range(N_CORES):
        b, g = divmod(c, 4)
        sl = slice(g * M, (g + 1) * M)
        ctxu = results[c]["ctxu"]  # [4, 65, S]
        ctx = (ctxu[:, :HD, :] / ctxu[:, HD:HD + 1, :]).transpose(2, 0, 1)
        out[b, :, sl] = ctx.reshape(S, M) + bv[sl][None, :]
    return out


def kernel(hidden_states, causal_bias, Wq, bq, Wk, bk, Wv, bv):
    nc = build_nc()
    in_maps = make_in_maps(hidden_states, causal_bias, Wq, bq, Wk, bk, Wv, bv)
    res = bass_utils.run_bass_kernel_spmd(nc, in_maps, core_ids=list(range(N_CORES)))
    return gather_output(res.results, bv)


# revision 12
# speedup vs baseline: 247.2824x; 247.2824x over previous
"""BertSelfAttention on 8 Trainium2 NeuronCores (Bass/Tile, SPMD, no collectives).

Problem: hidden_states [2, 2048, 1024], 16 heads x 64 dims, causal_bias added
along the key axis before softmax.

Sharding: core c handles batch b = c//4 and head-group g = c%4 (4 heads, i.e.
256 of the 1024 projection dims).  Pure SPMD - every core runs the same
program on its own slice; the host does the (free) slicing / transposes and
the final gather.

Per-core device algorithm (all matmuls in fp32r = full-rate fp32):
  QT[m, s] = Wq_g @ hsT + bq   (m = 256 local head dims, s = 2048 positions)
  KT[m, s] = Wk_g @ hsT + bk
  V [s, m] = (hs @ Wv_g.T) * expb[s]   (expb = exp(causal_bias), no bv)
  per head h (2 row-packed pairs):
    sT[k, sq]  = KT_h.T @ QT_h          (scores transposed, k = key pos)
    P [k, sq]  = exp(sT * 0.125)        (bias folded in via expb; no max
                                         subtraction needed: |s/8| < ~3)
    ctxu[65, sq] += [V'_h | expb].T @ P (rows 0..63 = unnormalized ctx^T,
                                         row 64 = softmax denominator)
  DMA ctxu to DRAM.
Host: ctx = (ctxu[:64] / ctxu[64]).T + bv  and scatter into [B, S, H].

The exp(bias) folding works because softmax(s + cb)_k = exp(s_k)*exp(cb_k) /
sum_k' exp(s_k')*exp(cb_k'), so scaling V rows and the denominator by
exp(cb_k) is exactly the bias add.
"""

import numpy as np

import concourse.bass as bass  # noqa: F401  (bass types via tile/bacc)
import concourse.tile as tile
from concourse import bacc, bass_utils, mybir

F32 = mybir.dt.float32
F32R = mybir.dt.float32r
AF = mybir.ActivationFunctionType

B, S, H = 2, 2048, 1024
NH, HD = 16, 64
M = 256          # per-core projection dims (4 heads)
KC = H // 128    # 8 contraction chunks for the projections
ST = S // 128    # 16 key-position chunks
N_CORES = 8

_NC_CACHE = {}


def _attention_kernel(tc, reps=1):
    nc = tc.nc
    hsT = nc.dram_tensor("hsT", [H, S], F32R, kind="ExternalInput").ap()
    WqT = nc.dram_tensor("WqT", [H, M], F32R, kind="ExternalInput").ap()
    WkT = nc.dram_tensor("WkT", [H, M], F32R, kind="ExternalInput").ap()
    WvT = nc.dram_tensor("WvT", [H, M], F32R, kind="ExternalInput").ap()
    bq = nc.dram_tensor("bq", [128, 2], F32, kind="ExternalInput").ap()
    bk = nc.dram_tensor("bk", [128, 2], F32, kind="ExternalInput").ap()
    expb = nc.dram_tensor("expb", [128, ST], F32, kind="ExternalInput").ap()
    ctxu = nc.dram_tensor("ctxu", [4, HD + 1, S], F32, kind="ExternalOutput").ap()

    for _rep in range(reps):
      with (
        tc.tile_pool(name="const", bufs=1) as const,
        tc.tile_pool(name="big", bufs=1) as big,
      ):
        expb_sb = const.tile([128, ST], F32, tag="expb", name="expb")
        nc.sync.dma_start(out=expb_sb[:], in_=expb[:])
        bq_sb = const.tile([128, 2], F32, tag="bq", name="bq")
        nc.sync.dma_start(out=bq_sb[:], in_=bq[:])
        bk_sb = const.tile([128, 2], F32, tag="bk", name="bk")
        nc.sync.dma_start(out=bk_sb[:], in_=bk[:])
        ones_sb = const.tile([128, 4], F32, tag="ones", name="ones")
        nc.vector.memset(ones_sb[:], 1.0)

        hsT_t = []
        for k in range(KC):
            t = big.tile([128, S], F32R, tag=f"hsT{k}", name=f"hsT{k}")
            nc.sync.dma_start(out=t[:], in_=hsT[k * 128:(k + 1) * 128, :])
            hsT_t.append(t)
        wq_t, wk_t, wv_t = [], [], []
        for w_dram, lst, nm in ((WqT, wq_t, "wq"), (WkT, wk_t, "wk"), (WvT, wv_t, "wv")):
            for k in range(KC):
                t = big.tile([128, M], F32R, tag=f"{nm}{k}", name=f"{nm}{k}")
                nc.sync.dma_start(out=t[:], in_=w_dram[k * 128:(k + 1) * 128, :])
                lst.append(t)

        # Persistent projection outputs.
        QT = [big.tile([128, S], F32R, tag=f"QT{t}", name=f"QT{t}") for t in range(2)]
        KT = [big.tile([128, S], F32R, tag=f"KT{t}", name=f"KT{t}") for t in range(2)]
        # V' with exp(bias) column interleaved: per key chunk, 4 head blocks
        # of [64 scaled V dims | expb] = 260 columns.
        Vp = [big.tile([128, 4, HD + 1], F32R, tag=f"Vp{s}", name=f"Vp{s}") for s in range(ST)]

        with tc.tile_pool(name="pp", bufs=3, space="PSUM") as pp:

            def qk_proj(mt):
                for w_t, out_t, bias_sb in ((wq_t, QT, bq_sb), (wk_t, KT, bk_sb)):
                    for sc in range(4):
                        ps = pp.tile([128, 512], F32, tag="qk", name="qk")
                        for k in range(KC):
                            nc.tensor.matmul(
                                ps[:],
                                w_t[k][:, mt * 128:(mt + 1) * 128],
                                hsT_t[k][:, sc * 512:(sc + 1) * 512],
                                start=(k == 0),
                                stop=(k == KC - 1),
                            )
                        nc.vector.tensor_scalar_add(
                            out_t[mt][:, sc * 512:(sc + 1) * 512],
                            ps[:],
                            bias_sb[:, mt:mt + 1],
                        )

            def v_proj(st):
                ps = pp.tile([128, M], F32, tag="v", name="v", bufs=2)
                for k in range(KC):
                    nc.tensor.matmul(
                        ps[:],
                        hsT_t[k][:, st * 128:(st + 1) * 128],
                        wv_t[k][:],
                        start=(k == 0),
                        stop=(k == KC - 1),
                    )
                nc.vector.tensor_scalar_mul(
                    Vp[st][:, :, 0:HD],
                    ps[:].rearrange("p (h d) -> p h d", h=4),
                    expb_sb[:, st:st + 1],
                )
                nc.vector.tensor_scalar_mul(
                    Vp[st][:, :, HD:HD + 1],
                    ones_sb[:].rearrange("p (h d) -> p h d", h=4),
                    expb_sb[:, st:st + 1],
                )

            # Order matters for the scheduler's priorities: attention on head
            # pair 0 needs QT[0]/KT[0] and the first V' chunks, so emit those
            # first; pair-1 projections overlap with pair-0 attention.
            qk_proj(0)
            for st in range(4):
                v_proj(st)
            for st in range(4, ST):
                v_proj(st)
            qk_proj(1)

        # Attention, head-pair at a time (pair p = local heads 2p, 2p+1 living
        # on SBUF partitions 0-63 / 64-127 of QT[p]/KT[p] - row-packed on PE).
        with (
            tc.tile_pool(name="pt", bufs=2) as pt_pool,
            tc.tile_pool(name="cs", bufs=2) as cs_pool,
            tc.tile_pool(name="sc", bufs=1, space="PSUM") as sc_pool,
            tc.tile_pool(name="cx", bufs=2, space="PSUM") as cx_pool,
        ):
            for p in range(2):
                for sqc in range(4):
                    sq = slice(sqc * 512, (sqc + 1) * 512)
                    cA = cx_pool.tile([HD + 1, 512], F32, tag="cA", name="cA")
                    cB = cx_pool.tile([HD + 1, 512], F32, tag="cB", name="cB")
                    for kk in range(8):
                        sA = sc_pool.tile([128, 1024], F32, tag="sA", name="sA")
                        sB = sc_pool.tile([128, 1024], F32, tag="sB", name="sB")
                        for i in range(2):
                            kch = 2 * kk + i
                            ks = slice(kch * 128, (kch + 1) * 128)
                            nc.tensor.matmul(
                                sA[:, i * 512:(i + 1) * 512],
                                KT[p][0:64, ks],
                                QT[p][0:64, sq],
                            )
                            nc.tensor.matmul(
                                sB[:, i * 512:(i + 1) * 512],
                                KT[p][64:128, ks],
                                QT[p][64:128, sq],
                            )
                        pA = pt_pool.tile([128, 1024], F32R, tag="pA", name="pA")
                        pB = pt_pool.tile([128, 1024], F32R, tag="pB", name="pB")
                        nc.scalar.activation(pA[:], sA[:], AF.Exp, scale=0.125)
                        nc.scalar.activation(pB[:], sB[:], AF.Exp, scale=0.125)
                        for i in range(2):
                            kch = 2 * kk + i
                            flags = dict(
                                start=(kk == 0 and i == 0),
                                stop=(kk == 7 and i == 1),
                            )
                            nc.tensor.matmul(
                                cA[:],
                                Vp[kch][:, 2 * p, :],
                                pA[:, i * 512:(i + 1) * 512],
                                **flags,
                            )
                            nc.tensor.matmul(
                                cB[:],
                                Vp[kch][:, 2 * p + 1, :],
                                pB[:, i * 512:(i + 1) * 512],
                                **flags,
                            )
                    oA = cs_pool.tile([HD + 1, 512], F32, tag="oA", name="oA")
                    oB = cs_pool.tile([HD + 1, 512], F32, tag="oB", name="oB")
                    nc.vector.tensor_copy(oA[:], cA[:])
                    nc.vector.tensor_copy(oB[:], cB[:])
                    nc.sync.dma_start(out=ctxu[2 * p, :, sq], in_=oA[:])
                    nc.sync.dma_start(out=ctxu[2 * p + 1, :, sq], in_=oB[:])


def _head_slices(p):
    # Head h of pair p uses V' block index: pair 0 -> blocks 0,1; pair 1 -> 2,3.
    return 2 * p, 2 * p + 1


def build_nc(reps=1):
    if reps in _NC_CACHE:
        return _NC_CACHE[reps]
    nc = bacc.Bacc("TRN2", target_bir_lowering=False, debug=False)
    with tile.TileContext(nc) as tc:
        _attention_kernel(tc, reps=reps)
    nc.compile()
    _NC_CACHE[reps] = nc
    return nc


def make_in_maps(hidden_states, causal_bias, Wq, bq, Wk, bk, Wv, bv):
    hs = np.ascontiguousarray(np.asarray(hidden_states, dtype=np.float32))
    cb = np.asarray(causal_bias, dtype=np.float32)
    expb = np.exp(cb).reshape(ST, 128).T.copy()  # [128, ST]
    hsT = [np.ascontiguousarray(hs[b].T) for b in range(B)]
    in_maps = []
    for c in range(N_CORES):
        b, g = divmod(c, 4)
        sl = slice(g * M, (g + 1) * M)
        in_maps.append({
            "hsT": hsT[b],
            "WqT": np.ascontiguousarray(np.asarray(Wq, np.float32)[sl].T),
            "WkT": np.ascontiguousarray(np.asarray(Wk, np.float32)[sl].T),
            "WvT": np.ascontiguousarray(np.asarray(Wv, np.float32)[sl].T),
            "bq": np.asarray(bq, np.float32)[sl].reshape(2, 128).T.copy(),
            "bk": np.asarray(bk, np.float32)[sl].reshape(2, 128).T.copy(),
            "expb": expb,
        })
    return in_maps


def gather_output(results, bv):
    bv = np.asarray(bv, np.float32)
    out = np.empty((B, S, H), np.float32)
    for c in range(N_CORES):
        b, g = divmod(c, 4)
        sl = slice(g * M, (g + 1) * M)
        ctxu = results[c]["ctxu"]  # [4, 65, S]
        ctx = (ctxu[:, :HD, :] / ctxu[:, HD:HD + 1, :]).transpose(2, 0, 1)
        out[b, :, sl] = ctx.reshape(S, M) + bv[sl][None, :]
    return out


def kernel(hidden_states, causal_bias, Wq, bq, Wk, bk, Wv, bv):
    nc = build_nc()
    in_maps = make_in_maps(hidden_states, causal_bias, Wq, bq, Wk, bk, Wv, bv)
    res = bass_utils.run_bass_kernel_spmd(nc, in_maps, core_ids=list(range(N_CORES)))
    return gather_output(res.results, bv)
